# revision 11
# baseline (speedup 1.0000x reference)
"""Trainium2 Bass kernel for nn_MultiHeadAttention (B=4, S=2048, D=1024, H=16).

Sharding: 8 cores = 4 batches x 2 head-groups. Core c handles batch b=c//2,
heads [8g, 8g+8) with g=c%2 (feature slice e in [512g, 512g+512)).
Each core:
  1. Projects Q,K into [e, s] layout and V into [s, e] layout (f32r matmuls,
     full PE rate with ~fp22 multiply precision, fp32 accumulate).
  2. Causal attention per head-pair in scoresT [k, q] layout: softmax along
     the partition (k) axis is handled by appending a ones-column to V so the
     PV matmul also produces the row sums; normalization happens on the
     small [64, 512] output tiles. Upper-triangle blocks are skipped;
     diagonal blocks are masked with a single 128x128 triangular tile.
     The two heads of a pair use PE row-groups 0-63 / 64-127 so their
     score matmuls run concurrently.
  3. Partial output projection with the row-shard of wo, interleaved per
     q-tile so it overlaps the (ACT-bound) attention of later q-tiles.
Host sums the two partial outputs per batch and adds bo.
"""

import sys

if "/opt/trn_rl_repo" not in sys.path:
    sys.path.insert(0, "/opt/trn_rl_repo")

import numpy as np

B, S, D, H, DK = 4, 2048, 1024, 16, 64
E = 512            # per-core feature slice (8 heads)
NCORES = 8
ST = 512           # s-tile width (matmul moving free dim)
NST = S // ST      # 4
NDC = D // 128     # 8 contraction chunks for projections
NEC = E // 128     # 4 e-chunks for Q/K layout
NKC = S // 128     # 16 k-chunks
HPC = 8            # heads per core

_CACHE = {}



def pv_emit(nc, ps_o, po, Vh, hp, item, qt, nkc):
    """Emit the PV matmuls for one drained kc, narrowed at the diagonal."""
    et, kc = item
    j = kc - 4 * qt
    c0 = 0 if j < 1 else (128 if j == 1 else 256)  # first column PV needs
    for u in range(2):
        base = u * 512
        nc.tensor.matmul(
            po[u][0:65, c0:512],
            Vh[:, kc, 2 * hp + u, :],
            et[:, base + c0 : base + 512],
            start=(kc == 0),
            stop=(kc == nkc - 1),
        )


def _build_nc():
    import concourse.mybir as mybir
    import concourse.tile as tile
    from concourse import bacc

    f32 = mybir.dt.float32
    f32r = mybir.dt.float32r
    AF = mybir.ActivationFunctionType

    nc = bacc.Bacc("TRN2", target_bir_lowering=False, debug=False)

    xqT = nc.dram_tensor("xqT", [D, S], f32r, kind="ExternalInput")
    xkT = nc.dram_tensor("xkT", [D, S], f32r, kind="ExternalInput")
    xvT = nc.dram_tensor("xvT", [D, S], f32r, kind="ExternalInput")
    wqT = nc.dram_tensor("wqT", [D, E], f32r, kind="ExternalInput")
    wkT = nc.dram_tensor("wkT", [D, E], f32r, kind="ExternalInput")
    wvT = nc.dram_tensor("wvT", [D, E], f32r, kind="ExternalInput")
    bqr = nc.dram_tensor("bqr", [128, NEC], f32, kind="ExternalInput")
    bkr = nc.dram_tensor("bkr", [128, NEC], f32, kind="ExternalInput")
    bvf = nc.dram_tensor("bvf", [1, E], f32r, kind="ExternalInput")
    woT = nc.dram_tensor("woT", [E, D], f32r, kind="ExternalInput")
    tri_d = nc.dram_tensor("tri", [128, 128], f32, kind="ExternalInput")
    onesd = nc.dram_tensor("onesd", [128, HPC], f32r, kind="ExternalInput")
    pout = nc.dram_tensor("pout", [S, D], f32, kind="ExternalOutput")

    with tile.TileContext(nc) as tc:
        with (
            tc.tile_pool(name="persist", bufs=1) as persist,
            tc.tile_pool(name="xt", bufs=2) as xt_pool,
            tc.tile_pool(name="w", bufs=1) as w_pool,
            tc.tile_pool(name="work", bufs=3) as work,
            tc.tile_pool(name="small", bufs=2) as small,
            tc.tile_pool(name="ps_s", bufs=2, space="PSUM") as ps_s,
            tc.tile_pool(name="ps_o", bufs=2, space="PSUM") as ps_o,
            tc.tile_pool(name="ps_p", bufs=2, space="PSUM") as ps_p,
        ):
            # ---- persistent tiles ----
            QhT = persist.tile([128, NEC, S], f32r, tag="QhT")  # later reused as attnT
            KhT = persist.tile([128, NEC, S], f32r, tag="KhT")
            Vh = persist.tile([128, NKC, HPC, DK + 1], f32r, tag="Vh")
            tri = persist.tile([128, 128], f32, tag="tri")
            bq_sb = persist.tile([128, NEC], f32, tag="bq_sb")
            bk_sb = persist.tile([128, NEC], f32, tag="bk_sb")
            bv_sb = persist.tile([1, E], f32r, tag="bv_sb")
            bv_bc = persist.tile([128, E], f32r, tag="bv_bc")
            on_sb = persist.tile([128, HPC], f32r, tag="on_sb")
            wo_sb = persist.tile([128, NEC, D], f32r, tag="wo_sb")

            # ---- constants ----
            nc.sync.dma_start(tri[:], tri_d[:])
            nc.sync.dma_start(on_sb[:], onesd[:])
            nc.sync.dma_start(bq_sb[:], bqr[:])
            nc.sync.dma_start(bk_sb[:], bkr[:])
            nc.sync.dma_start(bv_sb[:], bvf[:])
            nc.gpsimd.partition_broadcast(bv_bc[:], bv_sb[:])
            for kc in range(NKC):
                nc.vector.tensor_copy(out=Vh[:, kc, :, DK : DK + 1], in_=on_sb[:, :, None])

            # ---- Q/K projections (upfront): out[e, s] ----
            for x_d, w_d, b_sb, dst in (
                (xqT, wqT, bq_sb, QhT),
                (xkT, wkT, bk_sb, KhT),
            ):
                w_sb = w_pool.tile([128, NDC, E], f32r, tag="w")
                nc.sync.dma_start(w_sb[:], w_d.rearrange("(dc p) e -> p dc e", p=128))
                xr = x_d.rearrange("(dc p) s -> p dc s", p=128)
                for st in range(NST):
                    xt = xt_pool.tile([128, NDC, ST], f32r, tag="xt")
                    nc.sync.dma_start(xt[:], xr[:, :, st * ST : (st + 1) * ST])
                    for ec in range(NEC):
                        ps = ps_p.tile([128, ST], mybir.dt.float32, tag="pp")
                        for dc in range(NDC):
                            nc.tensor.matmul(
                                ps[:],
                                w_sb[:, dc, ec * 128 : (ec + 1) * 128],
                                xt[:, dc, :],
                                start=(dc == 0),
                                stop=(dc == NDC - 1),
                            )
                        nc.scalar.activation(
                            dst[:, ec, st * ST : (st + 1) * ST],
                            ps[:],
                            AF.Identity,
                            bias=b_sb[:, ec : ec + 1],
                        )

            # ---- per s-tile: V projection, then attention qt=st, then outproj ----
            w_sb = w_pool.tile([128, NDC, E], f32r, tag="w")
            nc.sync.dma_start(w_sb[:], wvT.rearrange("(dc p) e -> p dc e", p=128))
            xr = xvT.rearrange("(dc p) s -> p dc s", p=128)
            for st in range(NST):
                # V projection for this s-tile: out[s, e] (+ ones column)
                xt = xt_pool.tile([128, NDC, ST], f32r, tag="xt")
                nc.sync.dma_start(xt[:], xr[:, :, st * ST : (st + 1) * ST])
                for s4 in range(4):
                    sc = st * 4 + s4
                    ps = ps_p.tile([128, ST], mybir.dt.float32, tag="pp")
                    for dc in range(NDC):
                        nc.tensor.matmul(
                            ps[:],
                            xt[:, dc, s4 * 128 : (s4 + 1) * 128],
                            w_sb[:, dc, :],
                            start=(dc == 0),
                            stop=(dc == NDC - 1),
                        )
                    nc.vector.tensor_add(
                        out=Vh[:, sc, :, 0:DK],
                        in0=ps[:].rearrange("p (h e) -> p h e", h=HPC),
                        in1=bv_bc[:].rearrange("p (h e) -> p h e", h=HPC),
                    )
                if st == 0:
                    nc.sync.dma_start(
                        wo_sb[:], woT.rearrange("(dc p) e -> p dc e", p=128)
                    )

                # ---- attention for qt = st (head pairs share exp tiles) ----
                qt = st
                nkc = 4 * qt + 4
                for hp in range(4):
                    ec = hp
                    po = [
                        ps_o.tile([128, ST], mybir.dt.float32, tag="po", name=f"po{u}")
                        for u in range(2)
                    ]
                    pending = []
                    for kc in range(nkc):
                        psc = ps_s.tile([128, 2 * ST], mybir.dt.float32, tag="psc")
                        for u, r0 in ((0, 0), (1, 64)):
                            nc.tensor.matmul(
                                psc[:, u * ST : (u + 1) * ST],
                                KhT[r0 : r0 + 64, ec, kc * 128 : (kc + 1) * 128],
                                QhT[r0 : r0 + 64, ec, qt * ST : (qt + 1) * ST],
                                start=True,
                                stop=True,
                            )
                        et = work.tile([128, 2 * ST], f32r, tag="exp")
                        nc.scalar.activation(et[:], psc[:], AF.Exp, scale=0.125)
                        j = kc - 4 * qt
                        if j >= 0:
                            for u in range(2):
                                base = u * ST
                                if j == 3:
                                    # zero the 128 masked columns PV will read
                                    # (memset is not ISA-legal on f32r tiles)
                                    nc.vector.tensor_scalar_mul(
                                        et[:, base + 256 : base + 384],
                                        et[:, base + 256 : base + 384],
                                        0.0,
                                    )
                                nc.vector.tensor_mul(
                                    out=et[:, base + 128 * j : base + 128 * (j + 1)],
                                    in0=et[:, base + 128 * j : base + 128 * (j + 1)],
                                    in1=tri[:],
                                )
                        pending.append((et, kc))
                        if len(pending) > 2:
                            pv_emit(nc, ps_o, po, Vh, hp, pending.pop(0), qt, nkc)
                    while pending:
                        pv_emit(nc, ps_o, po, Vh, hp, pending.pop(0), qt, nkc)
                    # normalize: attnT[e, q] = po[e, q] * (1 / sums[q]);
                    # overwrite the consumed QhT region (QhT doubles as attnT)
                    for u, r0 in ((0, 0), (1, 64)):
                        rec = small.tile([1, ST], f32r, tag="rec")
                        with nc.allow_low_precision(reason="f32r holds fp32 bits"):
                            nc.vector.reciprocal(rec[:], po[u][64:65, :])
                        rb = small.tile([128, ST], f32r, tag="rb")
                        nc.gpsimd.partition_broadcast(rb[:], rec[:])
                        nc.vector.tensor_mul(
                            out=QhT[r0 : r0 + 64, ec, qt * ST : (qt + 1) * ST],
                            in0=po[u][0:64, :],
                            in1=rb[0:64, :],
                        )
                # ---- partial output projection for this qt's s-columns ----
                for mt in range(4 * qt, 4 * qt + 4):
                    ot = small.tile([128, D], f32, tag="ot", bufs=1)
                    for nt in range(2):
                        ps = ps_p.tile([128, ST], mybir.dt.float32, tag="pp")
                        for dc in range(NEC):
                            nc.tensor.matmul(
                                ps[:],
                                QhT[:, dc, mt * 128 : (mt + 1) * 128],
                                wo_sb[:, dc, nt * ST : (nt + 1) * ST],
                                start=(dc == 0),
                                stop=(dc == NEC - 1),
                            )
                        nc.vector.tensor_copy(out=ot[:, nt * ST : (nt + 1) * ST], in_=ps[:])
                    nc.sync.dma_start(pout[mt * 128 : (mt + 1) * 128, :], ot[:])

    nc.compile()
    return nc


def _get_nc():
    if "nc" not in _CACHE:
        _CACHE["nc"] = _build_nc()
    return _CACHE["nc"]


def prep_in_maps(q, k, v, wq, bq, wk, bk, wv, bv, wo):
    """Build the 8 per-core input dicts (host-side sharding)."""
    f = np.float32
    q = np.asarray(q, f).reshape(B, S, D)
    k = np.asarray(k, f).reshape(B, S, D)
    v = np.asarray(v, f).reshape(B, S, D)

    # triangular mask tile: allowed (1.0) iff kp <= qf
    kp = np.arange(128)[:, None]
    qf = np.arange(128)[None, :]
    tri = (kp <= qf).astype(f)

    xT = {}
    for b in range(B):
        xT[("q", b)] = np.ascontiguousarray(q[b].T)
        xT[("k", b)] = np.ascontiguousarray(k[b].T)
        xT[("v", b)] = np.ascontiguousarray(v[b].T)

    shard = {}
    for g in range(2):
        sl = slice(E * g, E * g + E)
        shard[("wqT", g)] = np.ascontiguousarray(np.asarray(wq, f)[sl, :].T)
        shard[("wkT", g)] = np.ascontiguousarray(np.asarray(wk, f)[sl, :].T)
        shard[("wvT", g)] = np.ascontiguousarray(np.asarray(wv, f)[sl, :].T)
        shard[("bqr", g)] = np.ascontiguousarray(np.asarray(bq, f)[sl].reshape(NEC, 128).T)
        shard[("bkr", g)] = np.ascontiguousarray(np.asarray(bk, f)[sl].reshape(NEC, 128).T)
        shard[("bvf", g)] = np.asarray(bv, f)[sl].reshape(1, E)
        shard[("woT", g)] = np.ascontiguousarray(np.asarray(wo, f).T[sl, :])

    in_maps = []
    for c in range(NCORES):
        b, g = c // 2, c % 2
        in_maps.append(
            {
                "xqT": xT[("q", b)],
                "xkT": xT[("k", b)],
                "xvT": xT[("v", b)],
                "wqT": shard[("wqT", g)],
                "wkT": shard[("wkT", g)],
                "wvT": shard[("wvT", g)],
                "bqr": shard[("bqr", g)],
                "bkr": shard[("bkr", g)],
                "bvf": shard[("bvf", g)],
                "woT": shard[("woT", g)],
                "tri": tri,
                "onesd": np.ones((128, HPC), f),
            }
        )
    return in_maps


def assemble(results, bo):
    """Sum head-group partials per batch, add bo."""
    bo = np.asarray(bo, np.float32)
    out = np.empty((B, S, D), np.float32)
    for b in range(B):
        out[b] = results[2 * b]["pout"] + results[2 * b + 1]["pout"] + bo
    return out


def kernel(q, k, v, mask, wq, bq, wk, bk, wv, bv, wo, bo):
    from concourse.bass_utils import run_bass_kernel_spmd

    nc = _get_nc()
    in_maps = prep_in_maps(q, k, v, wq, bq, wk, bk, wv, bv, wo)
    res = run_bass_kernel_spmd(nc, in_maps, list(range(NCORES)))
    return assemble(res.results, bo)


# revision 12
# speedup vs baseline: 54.1591x; 54.1591x over previous
"""Trainium2 Bass kernel for nn_MultiHeadAttention (B=4, S=2048, D=1024, H=16).

Sharding: 8 cores = 4 batches x 2 head-groups. Core c handles batch b=c//2,
heads [8g, 8g+8) with g=c%2 (feature slice e in [512g, 512g+512)).
Each core:
  1. Projects Q,K into [e, s] layout and V into [s, e] layout (f32r matmuls,
     full PE rate with ~fp22 multiply precision, fp32 accumulate).
  2. Causal attention per head-pair in scoresT [k, q] layout: softmax along
     the partition (k) axis is handled by appending a ones-column to V so the
     PV matmul also produces the row sums; normalization happens on the
     small [64, 512] output tiles. Upper-triangle blocks are skipped;
     diagonal blocks are masked with a single 128x128 triangular tile.
     The two heads of a pair use PE row-groups 0-63 / 64-127 so their
     score matmuls run concurrently.
  3. Partial output projection with the row-shard of wo, interleaved per
     q-tile so it overlaps the (ACT-bound) attention of later q-tiles.
Host sums the two partial outputs per batch and adds bo.
"""

import sys

if "/opt/trn_rl_repo" not in sys.path:
    sys.path.insert(0, "/opt/trn_rl_repo")

import numpy as np

B, S, D, H, DK = 4, 2048, 1024, 16, 64
E = 512            # per-core feature slice (8 heads)
NCORES = 8
ST = 512           # s-tile width (matmul moving free dim)
NST = S // ST      # 4
NDC = D // 128     # 8 contraction chunks for projections
NEC = E // 128     # 4 e-chunks for Q/K layout
NKC = S // 128     # 16 k-chunks
HPC = 8            # heads per core

_CACHE = {}



def pv_emit(nc, ps_o, po, Vh, hp, item, qt, nkc):
    """Emit the PV matmuls for one drained kc, narrowed at the diagonal."""
    et, kc = item
    j = kc - 4 * qt
    c0 = 0 if j < 1 else (128 if j == 1 else 256)  # first column PV needs
    for u in range(2):
        base = u * 512
        nc.tensor.matmul(
            po[u][0:65, c0:512],
            Vh[:, kc, 2 * hp + u, :],
            et[:, base + c0 : base + 512],
            start=(kc == 0),
            stop=(kc == nkc - 1),
        )


def _build_nc(loop_n=1):
    import contextlib
    import concourse.mybir as mybir
    import concourse.tile as tile
    from concourse import bacc

    f32 = mybir.dt.float32
    f32r = mybir.dt.float32r
    AF = mybir.ActivationFunctionType

    nc = bacc.Bacc("TRN2", target_bir_lowering=False, debug=False)

    xqT = nc.dram_tensor("xqT", [D, S], f32r, kind="ExternalInput")
    xkT = nc.dram_tensor("xkT", [D, S], f32r, kind="ExternalInput")
    xvT = nc.dram_tensor("xvT", [D, S], f32r, kind="ExternalInput")
    wqT = nc.dram_tensor("wqT", [D, E], f32r, kind="ExternalInput")
    wkT = nc.dram_tensor("wkT", [D, E], f32r, kind="ExternalInput")
    wvT = nc.dram_tensor("wvT", [D, E], f32r, kind="ExternalInput")
    bqr = nc.dram_tensor("bqr", [128, NEC], f32, kind="ExternalInput")
    bkr = nc.dram_tensor("bkr", [128, NEC], f32, kind="ExternalInput")
    bvf = nc.dram_tensor("bvf", [1, E], f32r, kind="ExternalInput")
    woT = nc.dram_tensor("woT", [E, D], f32r, kind="ExternalInput")
    tri_d = nc.dram_tensor("tri", [128, 128], f32, kind="ExternalInput")
    onesd = nc.dram_tensor("onesd", [128, HPC], f32r, kind="ExternalInput")
    pout = nc.dram_tensor("pout", [S, D], f32, kind="ExternalOutput")

    with tile.TileContext(nc) as tc:
        with (
            tc.tile_pool(name="persist", bufs=1) as persist,
            tc.tile_pool(name="xt", bufs=2) as xt_pool,
            tc.tile_pool(name="w", bufs=1) as w_pool,
            tc.tile_pool(name="work", bufs=3) as work,
            tc.tile_pool(name="small", bufs=2) as small,
            tc.tile_pool(name="ps_s", bufs=2, space="PSUM") as ps_s,
            tc.tile_pool(name="ps_o", bufs=2, space="PSUM") as ps_o,
            tc.tile_pool(name="ps_p", bufs=2, space="PSUM") as ps_p,
            tc.For_i(0, loop_n, 1) if loop_n > 1 else contextlib.nullcontext(),
        ):
            # ---- persistent tiles ----
            QhT = persist.tile([128, NEC, S], f32r, tag="QhT")  # later reused as attnT
            KhT = persist.tile([128, NEC, S], f32r, tag="KhT")
            Vh = persist.tile([128, NKC, HPC, DK + 1], f32r, tag="Vh")
            tri = persist.tile([128, 128], f32, tag="tri")
            bq_sb = persist.tile([128, NEC], f32, tag="bq_sb")
            bk_sb = persist.tile([128, NEC], f32, tag="bk_sb")
            bv_sb = persist.tile([1, E], f32r, tag="bv_sb")
            bv_bc = persist.tile([128, E], f32r, tag="bv_bc")
            on_sb = persist.tile([128, HPC], f32r, tag="on_sb")
            wo_sb = persist.tile([128, NEC, D], f32r, tag="wo_sb")

            # ---- constants ----
            nc.sync.dma_start(tri[:], tri_d[:])
            nc.sync.dma_start(on_sb[:], onesd[:])
            nc.sync.dma_start(bq_sb[:], bqr[:])
            nc.sync.dma_start(bk_sb[:], bkr[:])
            nc.sync.dma_start(bv_sb[:], bvf[:])
            nc.gpsimd.partition_broadcast(bv_bc[:], bv_sb[:])
            for kc in range(NKC):
                nc.vector.tensor_copy(out=Vh[:, kc, :, DK : DK + 1], in_=on_sb[:, :, None])

            # ---- Q/K projections (upfront): out[e, s] ----
            for x_d, w_d, b_sb, dst in (
                (xqT, wqT, bq_sb, QhT),
                (xkT, wkT, bk_sb, KhT),
            ):
                w_sb = w_pool.tile([128, NDC, E], f32r, tag="w")
                nc.sync.dma_start(w_sb[:], w_d.rearrange("(dc p) e -> p dc e", p=128))
                xr = x_d.rearrange("(dc p) s -> p dc s", p=128)
                for st in range(NST):
                    xt = xt_pool.tile([128, NDC, ST], f32r, tag="xt")
                    nc.sync.dma_start(xt[:], xr[:, :, st * ST : (st + 1) * ST])
                    for ec in range(NEC):
                        ps = ps_p.tile([128, ST], mybir.dt.float32, tag="pp")
                        for dc in range(NDC):
                            nc.tensor.matmul(
                                ps[:],
                                w_sb[:, dc, ec * 128 : (ec + 1) * 128],
                                xt[:, dc, :],
                                start=(dc == 0),
                                stop=(dc == NDC - 1),
                            )
                        nc.scalar.activation(
                            dst[:, ec, st * ST : (st + 1) * ST],
                            ps[:],
                            AF.Identity,
                            bias=b_sb[:, ec : ec + 1],
                        )

            # ---- per s-tile: V projection, then attention qt=st, then outproj ----
            w_sb = w_pool.tile([128, NDC, E], f32r, tag="w")
            nc.sync.dma_start(w_sb[:], wvT.rearrange("(dc p) e -> p dc e", p=128))
            xr = xvT.rearrange("(dc p) s -> p dc s", p=128)
            for st in range(NST):
                # V projection for this s-tile: out[s, e] (+ ones column)
                xt = xt_pool.tile([128, NDC, ST], f32r, tag="xt")
                nc.sync.dma_start(xt[:], xr[:, :, st * ST : (st + 1) * ST])
                for s4 in range(4):
                    sc = st * 4 + s4
                    ps = ps_p.tile([128, ST], mybir.dt.float32, tag="pp")
                    for dc in range(NDC):
                        nc.tensor.matmul(
                            ps[:],
                            xt[:, dc, s4 * 128 : (s4 + 1) * 128],
                            w_sb[:, dc, :],
                            start=(dc == 0),
                            stop=(dc == NDC - 1),
                        )
                    nc.vector.tensor_add(
                        out=Vh[:, sc, :, 0:DK],
                        in0=ps[:].rearrange("p (h e) -> p h e", h=HPC),
                        in1=bv_bc[:].rearrange("p (h e) -> p h e", h=HPC),
                    )
                if st == 0:
                    nc.sync.dma_start(
                        wo_sb[:], woT.rearrange("(dc p) e -> p dc e", p=128)
                    )

                # ---- attention for qt = st (head pairs share exp tiles) ----
                qt = st
                nkc = 4 * qt + 4
                for hp in range(4):
                    ec = hp
                    po = [
                        ps_o.tile([128, ST], mybir.dt.float32, tag="po", name=f"po{u}")
                        for u in range(2)
                    ]
                    pending = []
                    for kc in range(nkc):
                        psc = ps_s.tile([128, 2 * ST], mybir.dt.float32, tag="psc")
                        for u, r0 in ((0, 0), (1, 64)):
                            nc.tensor.matmul(
                                psc[:, u * ST : (u + 1) * ST],
                                KhT[r0 : r0 + 64, ec, kc * 128 : (kc + 1) * 128],
                                QhT[r0 : r0 + 64, ec, qt * ST : (qt + 1) * ST],
                                start=True,
                                stop=True,
                            )
                        et = work.tile([128, 2 * ST], f32r, tag="exp")
                        nc.scalar.activation(et[:], psc[:], AF.Exp, scale=0.125)
                        j = kc - 4 * qt
                        if j >= 0:
                            for u in range(2):
                                base = u * ST
                                if j == 3:
                                    # zero the 128 masked columns PV will read
                                    # (memset is not ISA-legal on f32r tiles)
                                    nc.vector.tensor_scalar_mul(
                                        et[:, base + 256 : base + 384],
                                        et[:, base + 256 : base + 384],
                                        0.0,
                                    )
                                nc.vector.tensor_mul(
                                    out=et[:, base + 128 * j : base + 128 * (j + 1)],
                                    in0=et[:, base + 128 * j : base + 128 * (j + 1)],
                                    in1=tri[:],
                                )
                        pending.append((et, kc))
                        if len(pending) > 2:
                            pv_emit(nc, ps_o, po, Vh, hp, pending.pop(0), qt, nkc)
                    while pending:
                        pv_emit(nc, ps_o, po, Vh, hp, pending.pop(0), qt, nkc)
                    # normalize: attnT[e, q] = po[e, q] * (1 / sums[q]);
                    # overwrite the consumed QhT region (QhT doubles as attnT)
                    for u, r0 in ((0, 0), (1, 64)):
                        rec = small.tile([1, ST], f32r, tag="rec")
                        with nc.allow_low_precision(reason="f32r holds fp32 bits"):
                            nc.vector.reciprocal(rec[:], po[u][64:65, :])
                        rb = small.tile([128, ST], f32r, tag="rb")
                        nc.gpsimd.partition_broadcast(rb[:], rec[:])
                        nc.vector.tensor_mul(
                            out=QhT[r0 : r0 + 64, ec, qt * ST : (qt + 1) * ST],
                            in0=po[u][0:64, :],
                            in1=rb[0:64, :],
                        )
                # ---- partial output projection for this qt's s-columns ----
                for mt in range(4 * qt, 4 * qt + 4):
                    ot = small.tile([128, D], f32, tag="ot", bufs=1)
                    for nt in range(2):
                        ps = ps_p.tile([128, ST], mybir.dt.float32, tag="pp")
                        for dc in range(NEC):
                            nc.tensor.matmul(
                                ps[:],
                                QhT[:, dc, mt * 128 : (mt + 1) * 128],
                                wo_sb[:, dc, nt * ST : (nt + 1) * ST],
                                start=(dc == 0),
                                stop=(dc == NEC - 1),
                            )
                        nc.vector.tensor_copy(out=ot[:, nt * ST : (nt + 1) * ST], in_=ps[:])
                    nc.sync.dma_start(pout[mt * 128 : (mt + 1) * 128, :], ot[:])

    nc.compile()
    return nc


def _get_nc(loop_n=1):
    key = ("nc", loop_n)
    if key not in _CACHE:
        _CACHE[key] = _build_nc(loop_n)
    return _CACHE[key]


def prep_in_maps(q, k, v, wq, bq, wk, bk, wv, bv, wo):
    """Build the 8 per-core input dicts (host-side sharding)."""
    f = np.float32
    q = np.asarray(q, f).reshape(B, S, D)
    k = np.asarray(k, f).reshape(B, S, D)
    v = np.asarray(v, f).reshape(B, S, D)

    # triangular mask tile: allowed (1.0) iff kp <= qf
    kp = np.arange(128)[:, None]
    qf = np.arange(128)[None, :]
    tri = (kp <= qf).astype(f)

    xT = {}
    for b in range(B):
        xT[("q", b)] = np.ascontiguousarray(q[b].T)
        xT[("k", b)] = np.ascontiguousarray(k[b].T)
        xT[("v", b)] = np.ascontiguousarray(v[b].T)

    shard = {}
    for g in range(2):
        sl = slice(E * g, E * g + E)
        shard[("wqT", g)] = np.ascontiguousarray(np.asarray(wq, f)[sl, :].T)
        shard[("wkT", g)] = np.ascontiguousarray(np.asarray(wk, f)[sl, :].T)
        shard[("wvT", g)] = np.ascontiguousarray(np.asarray(wv, f)[sl, :].T)
        shard[("bqr", g)] = np.ascontiguousarray(np.asarray(bq, f)[sl].reshape(NEC, 128).T)
        shard[("bkr", g)] = np.ascontiguousarray(np.asarray(bk, f)[sl].reshape(NEC, 128).T)
        shard[("bvf", g)] = np.asarray(bv, f)[sl].reshape(1, E)
        shard[("woT", g)] = np.ascontiguousarray(np.asarray(wo, f).T[sl, :])

    in_maps = []
    for c in range(NCORES):
        b, g = c // 2, c % 2
        in_maps.append(
            {
                "xqT": xT[("q", b)],
                "xkT": xT[("k", b)],
                "xvT": xT[("v", b)],
                "wqT": shard[("wqT", g)],
                "wkT": shard[("wkT", g)],
                "wvT": shard[("wvT", g)],
                "bqr": shard[("bqr", g)],
                "bkr": shard[("bkr", g)],
                "bvf": shard[("bvf", g)],
                "woT": shard[("woT", g)],
                "tri": tri,
                "onesd": np.ones((128, HPC), f),
            }
        )
    return in_maps


def assemble(results, bo):
    """Sum head-group partials per batch, add bo."""
    bo = np.asarray(bo, np.float32)
    out = np.empty((B, S, D), np.float32)
    for b in range(B):
        out[b] = results[2 * b]["pout"] + results[2 * b + 1]["pout"] + bo
    return out


def kernel(q, k, v, mask, wq, bq, wk, bk, wv, bv, wo, bo):
    from concourse.bass_utils import run_bass_kernel_spmd

    nc = _get_nc()
    in_maps = prep_in_maps(q, k, v, wq, bq, wk, bk, wv, bv, wo)
    res = run_bass_kernel_spmd(nc, in_maps, list(range(NCORES)))
    return assemble(res.results, bo)
